# revision 10
# baseline (speedup 1.0000x reference)
"""Trainium2 Bass kernel for a 3-layer GCN (directional, symmetric-norm,
self-loops, skip connections, LayerNorm between layers).

Strategy (8 NeuronCores, SPMD, single NEFF launch):
  - Nodes sharded by destination across 8 cores (12500 each, padded 12800).
  - Per layer, each core computes h' = (x @ W) * dinv[:, None] for its shard,
    stores it PACKED bf16 (64 feats = 128B/node; two nodes share a 256B
    "pair row"), and AllGathers each half-shard so every core has the full
    pair table (2 chunks x 25600 pairs x 256B).
  - Edges are bucketed by (dest core, source half/chunk, dest 128-window).
    Messages are fetched with GPSIMD dma_gather (256B pair rows, int16 pair
    indices < 25600), round-robined over 4 SWDGE queues so all four Q7
    descriptor-gen core pairs run in parallel.
  - Aggregation per 128-edge block: two one-hot matmuls on the TensorEngine
    (even-parity S @ msg[:, 0:64] and odd-parity S @ msg[:, 64:128]),
    accumulated in PSUM per (chunk, window), folded into an SBUF accumulator.
  - Epilogue (skip add, dinv scale, ReLU, LayerNorm) runs as a handful of
    whole-shard DVE ops using stride-0 broadcasts of per-node scalars.

Block counts per (chunk, window) are padded to the max over cores so one
program runs on all 8 cores with per-core data only.
"""

import math
import os
import sys

import numpy as np

for _p in ("/opt/trn_rl_repo",):
    if os.path.isdir(_p) and _p not in sys.path:
        sys.path.insert(0, _p)

import concourse.bacc as bacc
import concourse.bass as bass
import concourse.mybir as mybir
import concourse.tile as tile
from concourse.bass_utils import run_bass_kernel_spmd

try:
    from ml_dtypes import bfloat16 as np_bf16
except ImportError:  # pragma: no cover
    np_bf16 = mybir.dt.np(mybir.dt.bfloat16)
np_f8 = mybir.dt.np(mybir.dt.float8e4)

F32 = mybir.dt.float32
BF16 = mybir.dt.bfloat16
F8 = mybir.dt.float8e4
I16 = mybir.dt.int16
AOP = mybir.AluOpType
PAD_RR = 200.0


class Cfg:
    def __init__(self, N=100000, E=1600000, D=64, L=3, n_cores=8,
                 seg_edges=6144, sblk=16, eps=1e-5,
                 dma_scratch=16384, n_queues=4, force_single_packet=False):
        self.dma_scratch = dma_scratch
        self.n_queues = n_queues
        self.force_single_packet = force_single_packet
        assert N % n_cores == 0
        self.N, self.E, self.D, self.L, self.n_cores = N, E, D, L, n_cores
        self.eps = eps
        self.npc = N // n_cores                     # nodes per core
        nt0 = (self.npc + 127) // 128               # 128-node tiles (= windows)
        self.nt = ((nt0 + 1) // 2) * 2              # even number of tiles
        self.t_pad = self.nt * 128                  # padded shard size
        self.n_chunks = 2                           # source halves
        self.half_nodes = self.t_pad // 2           # nodes per half-shard
        self.chunk_pairs = self.half_nodes // 2 * n_cores  # pairs per chunk
        assert self.chunk_pairs <= 32767
        self.seg_edges = seg_edges                  # edges per dma_gather call
        assert seg_edges % 128 == 0
        self.segblk = seg_edges // 128
        self.sblk = sblk                            # S-tiles built per DVE op
        self.msg_dt = BF16
        self.row_elems = 128                        # pair row = 128 bf16 = 256B


def _prep(cfg, x, edge_index, Ws, bs, ln_g, ln_b):
    """Host-side preprocessing: degrees, edge bucketing, per-core arrays."""
    c = cfg
    row = np.asarray(edge_index[0], dtype=np.int64)
    col = np.asarray(edge_index[1], dtype=np.int64)
    deg = np.bincount(row, minlength=c.N).astype(np.float64) + 1.0
    dinv = (1.0 / np.sqrt(deg)).astype(np.float32)

    core = row // c.npc
    row_local = row - core * c.npc
    win = row_local >> 7
    row_rel = row_local & 127
    src_core = col // c.npc
    src_loc = col % c.npc
    half = src_loc // c.half_nodes              # source chunk (0/1)
    loc_in_half = src_loc - half * c.half_nodes
    pair_idx = src_core * (c.half_nodes // 2) + (loc_in_half >> 1)
    parity = (loc_in_half & 1).astype(np.int64)

    # bucket key and stable sort: (dest core, chunk, window)
    key = (core * c.n_chunks + half) * c.nt + win
    order = np.argsort(key, kind="stable")
    key_s = key[order]
    idx_s = pair_idx[order].astype(np.int32)
    rr_s = row_rel[order].astype(np.int32)
    par_s = parity[order]

    nbuck = c.n_cores * c.n_chunks * c.nt
    counts = np.bincount(key_s, minlength=nbuck).reshape(
        c.n_cores, c.n_chunks, c.nt)
    starts_flat = np.zeros(nbuck + 1, dtype=np.int64)
    np.cumsum(counts.reshape(-1), out=starts_flat[1:])

    # common (max over cores) block counts per (chunk, window)
    blocks = np.ceil(counts.max(axis=0) / 128.0).astype(np.int64)  # [2, nt]
    nblk_total = int(blocks.sum())
    e_pad = nblk_total * 128
    blk_off = np.zeros(c.n_chunks * c.nt + 1, dtype=np.int64)
    np.cumsum(blocks.reshape(-1), out=blk_off[1:])

    # per-core padded slot streams (pad: idx 0, rr sentinel both parities)
    idx_arrs, rre_arrs, rro_arrs = [], [], []
    for cc in range(c.n_cores):
        idx_a = np.zeros(e_pad, dtype=np.int16)
        rre_a = np.full(e_pad, PAD_RR, dtype=np.float64)
        rro_a = np.full(e_pad, PAD_RR, dtype=np.float64)
        for ck in range(c.n_chunks):
            for w in range(c.nt):
                b = starts_flat[(cc * c.n_chunks + ck) * c.nt + w]
                e = starts_flat[(cc * c.n_chunks + ck) * c.nt + w + 1]
                n = e - b
                o = blk_off[ck * c.nt + w] * 128
                idx_a[o:o + n] = idx_s[b:e].astype(np.int16)
                pe = par_s[b:e] == 0
                rre_a[o:o + n] = np.where(pe, rr_s[b:e], PAD_RR)
                rro_a[o:o + n] = np.where(pe, PAD_RR, rr_s[b:e])
        idx_arrs.append(idx_a)
        rre_arrs.append(rre_a)
        rro_arrs.append(rro_a)

    # wrap indices for dma_gather: flat i -> [i%16, i//16], replicated x8
    def wrap_idx(a):
        return np.tile(a.reshape(-1, 16).T, (8, 1)).astype(np.int16)

    # rr packed [128, nblk]: partition e%128, col = block
    def pack_rr(a):
        return a.reshape(nblk_total, 128).T.astype(np_bf16)

    per_core = []
    x = np.asarray(x, dtype=np.float32)
    for cc in range(c.n_cores):
        xs = np.zeros((c.t_pad, c.D), dtype=np.float32)
        xs[:c.npc] = x[cc * c.npc:(cc + 1) * c.npc]
        dl = np.zeros(c.t_pad, dtype=np.float32)
        dl[:c.npc] = dinv[cc * c.npc:(cc + 1) * c.npc]
        rr_eo = np.empty((128, 2 * nblk_total), dtype=np_bf16)
        rr_eo[:, 0::2] = pack_rr(rre_arrs[cc])
        rr_eo[:, 1::2] = pack_rr(rro_arrs[cc])
        per_core.append({
            "x_in": xs,
            "dinv_in": dl.reshape(c.nt, 128).T.copy(),      # [128, nt]
            "idxs_in": wrap_idx(idx_arrs[cc]),              # [128, e_pad//16]
            "rreo_in": rr_eo,                               # [128, 2*nblk]
        })

    consts = {
        "W_in": np.ascontiguousarray(np.asarray(Ws, dtype=np.float32)),
        "iota_in": np.tile(np.arange(128, dtype=np.float64),
                           (128, 2 * c.sblk)).astype(np_bf16),
        "i64_in": np.eye(64, dtype=np.float32),
        "i128_in": np.eye(128, dtype=np.float32),
    }
    bs = np.asarray(bs, dtype=np.float32)
    ln_g = np.asarray(ln_g, dtype=np.float32)
    ln_b = np.asarray(ln_b, dtype=np.float32)
    flags = {
        "bias": bool(np.any(bs != 0.0)),
        "affine": bool(np.any(ln_g != 1.0) or np.any(ln_b != 0.0)),
    }
    if flags["bias"]:
        consts["bs_in"] = np.tile(bs[:, None, :], (1, 128, 1))        # [L,128,64]
    if flags["affine"]:
        consts["lng_in"] = np.tile(ln_g[:, None, :], (1, 128, 1))     # [L-1,128,64]
        consts["lnb_in"] = np.tile(ln_b[:, None, :], (1, 128, 1))
    struct = {
        "blocks": blocks,            # [n_chunks, nt]
        "nblk_total": nblk_total,
        "e_pad": e_pad,
    }
    return per_core, consts, struct, flags


def _build(cfg, struct, flags):
    """Build the Bass/Tile program. Returns nc."""
    c = cfg
    blocks = struct["blocks"]
    nblk_total = struct["nblk_total"]
    e_pad = struct["e_pad"]
    D = c.D
    NT = c.nt
    HT = NT // 2                 # tiles per half
    MSG = c.msg_dt
    ROWE = c.row_elems

    # first chunk contributing each window (for copy-vs-add into agg)
    first_ck = [None] * NT
    for w in range(NT):
        for ck in range(c.n_chunks):
            if blocks[ck, w] > 0:
                first_ck[w] = ck
                break

    nc = bacc.Bacc("TRN2", num_devices=c.n_cores, target_bir_lowering=False,
                   debug=False, enable_asserts=False,
                   num_swdge_queues=c.n_queues,
                   dynamic_dma_scratch_size=c.dma_scratch)

    # I/O
    x_in = nc.dram_tensor("x_in", [c.t_pad, D], F32, kind="ExternalInput")
    dinv_in = nc.dram_tensor("dinv_in", [128, NT], F32, kind="ExternalInput")
    idxs_in = nc.dram_tensor("idxs_in", [128, e_pad // 16], I16, kind="ExternalInput")
    rreo_in = nc.dram_tensor("rreo_in", [128, 2 * nblk_total], MSG, kind="ExternalInput")
    iota_in = nc.dram_tensor("iota_in", [128, 2 * c.sblk * 128], MSG, kind="ExternalInput")
    W_in = nc.dram_tensor("W_in", [c.L, D, D], F32, kind="ExternalInput")
    i64_in = nc.dram_tensor("i64_in", [64, 64], F32, kind="ExternalInput")
    i128_in = nc.dram_tensor("i128_in", [128, 128], F32, kind="ExternalInput")
    if flags["bias"]:
        bs_in = nc.dram_tensor("bs_in", [c.L, 128, D], F32, kind="ExternalInput")
    if flags["affine"]:
        lng_in = nc.dram_tensor("lng_in", [c.L - 1, 128, D], F32, kind="ExternalInput")
        lnb_in = nc.dram_tensor("lnb_in", [c.L - 1, 128, D], F32, kind="ExternalInput")
    emb_out = nc.dram_tensor("emb_out", [c.t_pad, D], F32, kind="ExternalOutput")
    x_out = nc.dram_tensor("x_out", [c.t_pad, D], F32, kind="ExternalOutput")

    with tile.TileContext(nc) as tc:
        with (
            tc.tile_pool(name="dram", bufs=1, space="DRAM") as dram_pool,
            tc.tile_pool(name="const", bufs=1) as const_pool,
            tc.tile_pool(name="state", bufs=1) as state_pool,
            tc.tile_pool(name="xT", bufs=2) as xT_pool,
            tc.tile_pool(name="msg", bufs=4) as msg_pool,
            tc.tile_pool(name="iseg", bufs=4) as iseg_pool,
            tc.tile_pool(name="sS", bufs=4) as s_pool,
            tc.tile_pool(name="stats", bufs=2) as stats_pool,
            tc.tile_pool(name="pagg", bufs=4, space="PSUM") as pagg_pool,
            tc.tile_pool(name="pbig", bufs=2, space="PSUM") as pbig_pool,
            tc.tile_pool(name="ptr", bufs=2, space="PSUM") as ptr_pool,
        ):
            # ---- DRAM internal buffers: packed bf16 shard + gathered tables
            hp_shards = [
                dram_pool.tile([c.t_pad, D], MSG, name=f"hp_shard{i}")
                for i in range(c.L)]
            hp_fulls = [
                [dram_pool.tile([c.chunk_pairs, ROWE], MSG, addr_space="Shared",
                                name=f"hp_full{i}q{q}")
                 for q in range(c.n_chunks)]
                for i in range(c.L)]

            # ---- constants ----
            dinvT = const_pool.tile([128, NT], F32)
            nc.sync.dma_start(dinvT[:], dinv_in[:])
            rrEO = const_pool.tile([128, 2 * nblk_total], MSG)
            nc.sync.dma_start(rrEO[:], rreo_in[:])
            iota16 = const_pool.tile([128, 2 * c.sblk, 128], MSG)
            nc.sync.dma_start(iota16[:], iota_in[:].rearrange("p (s k) -> p s k", k=128))
            i64 = const_pool.tile([64, 64], F32)
            nc.sync.dma_start(i64[:], i64_in[:])
            i128 = const_pool.tile([128, 128], F32)
            nc.sync.dma_start(i128[:], i128_in[:])
            eps_sb = const_pool.tile([128, 1], F32)
            nc.vector.memset(eps_sb[:], float(c.eps))
            W_sb = const_pool.tile([64, c.L, D], F32)
            nc.sync.dma_start(W_sb[:], W_in[:].rearrange("l p j -> p l j"))
            if flags["bias"]:
                bs_sb = const_pool.tile([128, c.L, D], F32)
                nc.sync.dma_start(bs_sb[:], bs_in[:].rearrange("l p j -> p l j"))
            if flags["affine"]:
                lng_sb = const_pool.tile([128, c.L - 1, D], F32)
                nc.sync.dma_start(lng_sb[:], lng_in[:].rearrange("l p j -> p l j"))
                lnb_sb = const_pool.tile([128, c.L - 1, D], F32)
                nc.sync.dma_start(lnb_sb[:], lnb_in[:].rearrange("l p j -> p l j"))

            # ---- persistent state ----
            x_state = state_pool.tile([128, NT, D], F32)
            hp_own = state_pool.tile([128, NT, D], F32)
            hp_bf = state_pool.tile([128, NT, D], MSG)
            agg = state_pool.tile([128, NT, D], F32)

            nc.sync.dma_start(x_state[:], x_in[:].rearrange("(t p) f -> p t f", p=128))

            dinv_bc = dinvT[:].broadcast_to([128, NT, D])

            gq = [0]  # round-robin gather queue counter
            pending_ags = []  # deferred collective dispatches [fn, fire_at]

            def emit_linear(layer, h):
                """h' = (x @ W) * dinv for half h; cast, store, all-gather."""
                t0 = h * HT
                st = t0
                while st < t0 + HT:
                    n_t = min(4, t0 + HT - st)
                    xdT = xT_pool.tile([64, 4, 128], F32, tag="xdT")
                    for j in range(n_t):
                        t = st + j
                        ptr = ptr_pool.tile([64, 128], F32, tag="ptr")
                        nc.tensor.transpose(ptr[:], x_state[:, t, :], i128[:])
                        nc.scalar.copy(xdT[:, j, :], ptr[:])
                    hT_ps = pbig_pool.tile([64, 4 * 128], F32)
                    nc.tensor.matmul(
                        hT_ps[:, :n_t * 128],
                        W_sb[:, layer, :],
                        xdT[:, :n_t, :],
                        start=True, stop=True)
                    hT_sb = xT_pool.tile([64, 4, 128], F32, tag="hT")
                    nc.scalar.copy(
                        hT_sb[:, :n_t, :],
                        hT_ps[:, :n_t * 128].rearrange("p (a b) -> p a b", b=128))
                    for j in range(n_t):
                        t = st + j
                        ptr2 = ptr_pool.tile([128, 64], F32, tag="ptr")
                        nc.tensor.transpose(ptr2[:], hT_sb[:, j, :], i64[:])
                        nc.vector.tensor_scalar(
                            hp_own[:, t, :], ptr2[:], dinvT[:, t:t + 1], None,
                            AOP.mult)
                    st += n_t
                nc.vector.tensor_copy(
                    hp_bf[:, t0:t0 + HT, :], hp_own[:, t0:t0 + HT, :])
                nc.sync.dma_start(
                    hp_shards[layer][:].rearrange(
                        "(t p) f -> p t f", p=128)[:, t0:t0 + HT, :],
                    hp_bf[:, t0:t0 + HT, :])
                def _ag(layer=layer, h=h):
                    nc.gpsimd.collective_compute(
                        "AllGather", AOP.bypass,
                        replica_groups=[list(range(c.n_cores))],
                        ins=[hp_shards[layer][
                            h * c.half_nodes:(h + 1) * c.half_nodes, :].opt()],
                        outs=[hp_fulls[layer][h][:].opt()],
                    )
                if layer == 0 and h == 0:
                    _ag()          # nothing ahead of it in the Pool queue
                else:
                    pending_ags.append([_ag, gq[0] + 6])

            def emit_epilogue(layer, h):
                """skip add + dinv scale + relu (+LN) for half h windows."""
                t0, t1 = h * HT, (h + 1) * HT
                a = agg[:, t0:t1, :]
                xs = x_state[:, t0:t1, :]
                db = dinvT[:, t0:t1].broadcast_to([128, HT, D])
                nc.vector.tensor_tensor(a, a, hp_own[:, t0:t1, :], AOP.add)
                nc.vector.tensor_tensor(a, a, db, AOP.mult)
                nc.vector.tensor_tensor(xs, xs, a, AOP.add)
                if flags["bias"]:
                    for t in range(t0, t1):
                        nc.vector.tensor_tensor(
                            x_state[:, t, :], x_state[:, t, :],
                            bs_sb[:, layer, :], AOP.add)
                if layer == c.L - 1:
                    # emb = x (pre-relu); x_out = relu(emb)
                    nc.sync.dma_start(
                        emb_out[:].rearrange(
                            "(t p) f -> p t f", p=128)[:, t0:t1, :], xs)
                    nc.vector.tensor_scalar(a, xs, 0.0, None, AOP.max)
                    nc.sync.dma_start(
                        x_out[:].rearrange(
                            "(t p) f -> p t f", p=128)[:, t0:t1, :], a)
                else:
                    nc.vector.tensor_scalar(xs, xs, 0.0, None, AOP.max)
                    mu = stats_pool.tile([128, HT], F32, tag="mu")
                    ss = stats_pool.tile([128, HT], F32, tag="ss")
                    rstd = stats_pool.tile([128, HT], F32, tag="rstd")
                    vtmp = stats_pool.tile([128, HT], F32, tag="vtmp")
                    nc.vector.tensor_reduce(mu[:], xs, mybir.AxisListType.X, AOP.add)
                    nc.scalar.activation(a, xs, mybir.ActivationFunctionType.Square)
                    nc.vector.tensor_reduce(ss[:], a, mybir.AxisListType.X, AOP.add)
                    nc.vector.tensor_scalar(mu[:], mu[:], 1.0 / D, None, AOP.mult)
                    nc.vector.tensor_tensor(vtmp[:], mu[:], mu[:], AOP.mult)
                    nc.vector.scalar_tensor_tensor(
                        vtmp[:], ss[:], 1.0 / D, vtmp[:], AOP.mult, AOP.subtract)
                    nc.scalar.activation(vtmp[:], vtmp[:],
                                         mybir.ActivationFunctionType.Sqrt,
                                         bias=eps_sb[:])
                    nc.vector.reciprocal(rstd[:], vtmp[:])
                    nc.vector.tensor_tensor(
                        xs, xs, mu[:].broadcast_to([128, HT, D]), AOP.subtract)
                    nc.vector.tensor_tensor(
                        xs, xs, rstd[:].broadcast_to([128, HT, D]), AOP.mult)
                    if flags["affine"]:
                        for t in range(t0, t1):
                            nc.vector.tensor_tensor(
                                x_state[:, t, :], x_state[:, t, :],
                                lng_sb[:, layer, :], AOP.mult)
                            nc.vector.tensor_tensor(
                                x_state[:, t, :], x_state[:, t, :],
                                lnb_sb[:, layer, :], AOP.add)

            nidx_regs = {}

            emit_linear(0, 0)
            emit_linear(0, 1)

            for layer in range(c.L):
                # windows with no edges at all
                for w in range(NT):
                    if first_ck[w] is None:
                        nc.vector.memset(agg[:, w, :], 0.0)

                # ---- gather + one-hot matmul reduction ----
                gb = 0               # global block id
                sEO = None
                ps = None
                done_h0 = False

                def boundary_work():
                    emit_epilogue(layer, 0)
                    if layer < c.L - 1:
                        emit_linear(layer + 1, 0)

                for ck in range(c.n_chunks):
                    ck_blocks = []   # (w, idx_in_group, group_size)
                    for w in range(NT):
                        for i in range(int(blocks[ck, w])):
                            ck_blocks.append((w, i, int(blocks[ck, w])))
                    nb_ck = len(ck_blocks)
                    if nb_ck == 0:
                        continue
                    in_rows = hp_fulls[layer][ck][:]
                    n_seg = (nb_ck + c.segblk - 1) // c.segblk
                    gb0_ck = gb
                    for s in range(n_seg):
                        b0 = s * c.segblk
                        nblk_s = min(c.segblk, nb_ck - b0)
                        nidx = nblk_s * 128
                        goff = (gb0_ck + b0) * 8   # idx cols (8 per 128 slots)
                        iseg = iseg_pool.tile([128, c.segblk * 8], I16)
                        nc.sync.dma_start(
                            iseg[:, :nblk_s * 8],
                            idxs_in[:, goff:goff + nblk_s * 8])
                        msg = msg_pool.tile([128, c.segblk, ROWE], MSG)
                        if nidx not in nidx_regs:
                            nidx_regs[nidx] = nc.gpsimd.to_reg(nidx)
                        nc.gpsimd.dma_gather(
                            msg[:, :nblk_s, :], in_rows,
                            iseg[:, :nblk_s * 8],
                            nidx, nidx_regs[nidx], ROWE,
                            single_packet=(nidx <= 1024 or c.force_single_packet),
                            queue_num=gq[0] % c.n_queues)
                        gq[0] += 1
                        while pending_ags and gq[0] >= pending_ags[0][1]:
                            pending_ags.pop(0)[0]()
                        for bl in range(nblk_s):
                            w, gi, gsz = ck_blocks[b0 + bl]
                            # h0 windows all finalized -> overlap boundary work
                            if ck == c.n_chunks - 1 and w >= HT and not done_h0:
                                boundary_work()
                                done_h0 = True
                            g = gb + b0 + bl
                            # build S tiles (even|odd interleaved) per sblk
                            if g % c.sblk == 0:
                                k = min(c.sblk, nblk_total - g)
                                sEO = s_pool.tile([128, c.sblk, 2, 128], MSG,
                                                  tag="SEO")
                                nc.vector.tensor_tensor(
                                    sEO[:, :k, :, :].rearrange("p a b c -> p (a b) c"),
                                    iota16[:, :2 * k, :],
                                    rrEO[:, 2 * g:2 * (g + k)].broadcast_to(
                                        [128, 2 * k, 128]),
                                    AOP.is_equal)
                            if gi == 0:
                                ps = pagg_pool.tile([128, D], F32)
                            nc.tensor.matmul(
                                ps[:], sEO[:, g % c.sblk, 0, :],
                                msg[:, bl, 0:64],
                                start=(gi == 0), stop=False)
                            nc.tensor.matmul(
                                ps[:], sEO[:, g % c.sblk, 1, :],
                                msg[:, bl, 64:128],
                                start=False, stop=(gi == gsz - 1))
                            if gi == gsz - 1:
                                if first_ck[w] == ck:
                                    nc.scalar.copy(agg[:, w, :], ps[:])
                                else:
                                    nc.vector.tensor_tensor(
                                        agg[:, w, :], agg[:, w, :], ps[:], AOP.add)
                    gb += nb_ck

                while pending_ags:
                    pending_ags.pop(0)[0]()
                if not done_h0:
                    boundary_work()
                emit_epilogue(layer, 1)
                if layer < c.L - 1:
                    emit_linear(layer + 1, 1)

    nc.compile()
    return nc


_CACHE = {}
last_results = None


def _run(cfg, inputs, trace=False):
    global last_results
    per_core, consts, struct, flags = _prep(cfg, **inputs)
    key = (cfg.N, cfg.E, cfg.seg_edges, cfg.n_queues, struct["nblk_total"])
    if key not in _CACHE:
        _CACHE[key] = _build(cfg, struct, flags)
    nc = _CACHE[key]
    in_maps = []
    for cc in range(cfg.n_cores):
        m = dict(consts)
        m.update(per_core[cc])
        in_maps.append(m)
    res = run_bass_kernel_spmd(nc, in_maps, list(range(cfg.n_cores)), trace=trace)
    last_results = res
    emb = np.concatenate(
        [np.asarray(r["emb_out"])[:cfg.npc] for r in res.results], axis=0)
    xf = np.concatenate(
        [np.asarray(r["x_out"])[:cfg.npc] for r in res.results], axis=0)
    return emb, xf


def kernel(x, edge_index, Ws, bs, ln_g, ln_b):
    cfg = Cfg(seg_edges=int(os.environ.get("GCN_SEG", "6144")),
              sblk=int(os.environ.get("GCN_SBLK", "16")),
              dma_scratch=int(os.environ.get("GCN_SCRATCH", "16384")),
              n_queues=int(os.environ.get("GCN_NQ", "4")),
              force_single_packet=os.environ.get("GCN_SP", "0") == "1")
    return _run(cfg, dict(x=x, edge_index=edge_index, Ws=Ws, bs=bs,
                          ln_g=ln_g, ln_b=ln_b),
                trace=os.environ.get("GCN_TRACE", "0") == "1")


# revision 11
# speedup vs baseline: 1.0078x; 1.0078x over previous
"""Trainium2 Bass kernel for a 3-layer GCN (directional, symmetric-norm,
self-loops, skip connections, LayerNorm between layers).

Strategy (8 NeuronCores, SPMD, single NEFF launch):
  - Nodes sharded by destination across 8 cores (12500 each, padded 12800).
  - Per layer, each core computes h' = (x @ W) * dinv[:, None] for its shard,
    stores it PACKED bf16 (64 feats = 128B/node; two nodes share a 256B
    "pair row"), and AllGathers each half-shard so every core has the full
    pair table (2 chunks x 25600 pairs x 256B).
  - Edges are bucketed by (dest core, source half/chunk, dest 128-window).
    Messages are fetched with GPSIMD dma_gather (256B pair rows, int16 pair
    indices < 25600), round-robined over 4 SWDGE queues so all four Q7
    descriptor-gen core pairs run in parallel.
  - Aggregation per 128-edge block: two one-hot matmuls on the TensorEngine
    (even-parity S @ msg[:, 0:64] and odd-parity S @ msg[:, 64:128]),
    accumulated in PSUM per (chunk, window), folded into an SBUF accumulator.
  - Epilogue (skip add, dinv scale, ReLU, LayerNorm) runs as a handful of
    whole-shard DVE ops using stride-0 broadcasts of per-node scalars.

Block counts per (chunk, window) are padded to the max over cores so one
program runs on all 8 cores with per-core data only.
"""

import math
import os
import sys

import numpy as np

for _p in ("/opt/trn_rl_repo",):
    if os.path.isdir(_p) and _p not in sys.path:
        sys.path.insert(0, _p)

import concourse.bacc as bacc
import concourse.bass as bass
import concourse.mybir as mybir
import concourse.tile as tile
from concourse.bass_utils import run_bass_kernel_spmd

try:
    from ml_dtypes import bfloat16 as np_bf16
except ImportError:  # pragma: no cover
    np_bf16 = mybir.dt.np(mybir.dt.bfloat16)
np_f8 = mybir.dt.np(mybir.dt.float8e4)

F32 = mybir.dt.float32
BF16 = mybir.dt.bfloat16
F8 = mybir.dt.float8e4
I16 = mybir.dt.int16
AOP = mybir.AluOpType
PAD_RR = 200.0


class Cfg:
    def __init__(self, N=100000, E=1600000, D=64, L=3, n_cores=8,
                 seg_edges=3072, sblk=8, eps=1e-5,
                 dma_scratch=16384, n_queues=4, force_single_packet=False):
        self.dma_scratch = dma_scratch
        self.n_queues = n_queues
        self.force_single_packet = force_single_packet
        assert N % n_cores == 0
        self.N, self.E, self.D, self.L, self.n_cores = N, E, D, L, n_cores
        self.eps = eps
        self.npc = N // n_cores                     # nodes per core
        nt0 = (self.npc + 127) // 128               # 128-node tiles (= windows)
        self.nt = ((nt0 + 1) // 2) * 2              # even number of tiles
        self.t_pad = self.nt * 128                  # padded shard size
        self.n_chunks = 2                           # source halves
        self.half_nodes = self.t_pad // 2           # nodes per half-shard
        self.chunk_pairs = self.half_nodes // 2 * n_cores  # pairs per chunk
        assert self.chunk_pairs <= 32767
        self.seg_edges = seg_edges                  # edges per dma_gather call
        assert seg_edges % 128 == 0
        self.segblk = seg_edges // 128
        self.sblk = sblk                            # S-tiles built per DVE op
        self.msg_dt = BF16
        self.row_elems = 128                        # pair row = 128 bf16 = 256B


def _prep(cfg, x, edge_index, Ws, bs, ln_g, ln_b):
    """Host-side preprocessing: degrees, edge bucketing, per-core arrays."""
    c = cfg
    row = np.asarray(edge_index[0], dtype=np.int64)
    col = np.asarray(edge_index[1], dtype=np.int64)
    deg = np.bincount(row, minlength=c.N).astype(np.float64) + 1.0
    dinv = (1.0 / np.sqrt(deg)).astype(np.float32)

    core = row // c.npc
    row_local = row - core * c.npc
    win = row_local >> 7
    row_rel = row_local & 127
    src_core = col // c.npc
    src_loc = col % c.npc
    half = src_loc // c.half_nodes              # source chunk (0/1)
    loc_in_half = src_loc - half * c.half_nodes
    pair_idx = src_core * (c.half_nodes // 2) + (loc_in_half >> 1)
    parity = (loc_in_half & 1).astype(np.int64)

    # bucket key and stable sort: (dest core, chunk, window)
    key = (core * c.n_chunks + half) * c.nt + win
    order = np.argsort(key, kind="stable")
    key_s = key[order]
    idx_s = pair_idx[order].astype(np.int32)
    rr_s = row_rel[order].astype(np.int32)
    par_s = parity[order]

    nbuck = c.n_cores * c.n_chunks * c.nt
    counts = np.bincount(key_s, minlength=nbuck).reshape(
        c.n_cores, c.n_chunks, c.nt)
    starts_flat = np.zeros(nbuck + 1, dtype=np.int64)
    np.cumsum(counts.reshape(-1), out=starts_flat[1:])

    # common (max over cores) block counts per (chunk, window)
    blocks = np.ceil(counts.max(axis=0) / 128.0).astype(np.int64)  # [2, nt]
    nblk_total = int(blocks.sum())
    e_pad = nblk_total * 128
    blk_off = np.zeros(c.n_chunks * c.nt + 1, dtype=np.int64)
    np.cumsum(blocks.reshape(-1), out=blk_off[1:])

    # per-core padded slot streams (pad: idx 0, rr sentinel both parities)
    idx_arrs, rre_arrs, rro_arrs = [], [], []
    for cc in range(c.n_cores):
        idx_a = np.zeros(e_pad, dtype=np.int16)
        rre_a = np.full(e_pad, PAD_RR, dtype=np.float64)
        rro_a = np.full(e_pad, PAD_RR, dtype=np.float64)
        for ck in range(c.n_chunks):
            for w in range(c.nt):
                b = starts_flat[(cc * c.n_chunks + ck) * c.nt + w]
                e = starts_flat[(cc * c.n_chunks + ck) * c.nt + w + 1]
                n = e - b
                o = blk_off[ck * c.nt + w] * 128
                idx_a[o:o + n] = idx_s[b:e].astype(np.int16)
                pe = par_s[b:e] == 0
                rre_a[o:o + n] = np.where(pe, rr_s[b:e], PAD_RR)
                rro_a[o:o + n] = np.where(pe, PAD_RR, rr_s[b:e])
        idx_arrs.append(idx_a)
        rre_arrs.append(rre_a)
        rro_arrs.append(rro_a)

    # wrap indices for dma_gather: flat i -> [i%16, i//16], replicated x8
    def wrap_idx(a):
        return np.tile(a.reshape(-1, 16).T, (8, 1)).astype(np.int16)

    # rr packed [128, nblk]: partition e%128, col = block
    def pack_rr(a):
        return a.reshape(nblk_total, 128).T.astype(np_bf16)

    per_core = []
    x = np.asarray(x, dtype=np.float32)
    for cc in range(c.n_cores):
        xs = np.zeros((c.t_pad, c.D), dtype=np.float32)
        xs[:c.npc] = x[cc * c.npc:(cc + 1) * c.npc]
        dl = np.zeros(c.t_pad, dtype=np.float32)
        dl[:c.npc] = dinv[cc * c.npc:(cc + 1) * c.npc]
        rr_eo = np.empty((128, 2 * nblk_total), dtype=np_bf16)
        rr_eo[:, 0::2] = pack_rr(rre_arrs[cc])
        rr_eo[:, 1::2] = pack_rr(rro_arrs[cc])
        per_core.append({
            "x_in": xs,
            "dinv_in": dl.reshape(c.nt, 128).T.copy(),      # [128, nt]
            "idxs_in": wrap_idx(idx_arrs[cc]),              # [128, e_pad//16]
            "rreo_in": rr_eo,                               # [128, 2*nblk]
        })

    consts = {
        "W_in": np.ascontiguousarray(np.asarray(Ws, dtype=np.float32)),
        "iota_in": np.tile(np.arange(128, dtype=np.float64),
                           (128, 2 * c.sblk)).astype(np_bf16),
        "i64_in": np.eye(64, dtype=np.float32),
        "i128_in": np.eye(128, dtype=np.float32),
    }
    bs = np.asarray(bs, dtype=np.float32)
    ln_g = np.asarray(ln_g, dtype=np.float32)
    ln_b = np.asarray(ln_b, dtype=np.float32)
    flags = {
        "bias": bool(np.any(bs != 0.0)),
        "affine": bool(np.any(ln_g != 1.0) or np.any(ln_b != 0.0)),
    }
    if flags["bias"]:
        consts["bs_in"] = np.tile(bs[:, None, :], (1, 128, 1))        # [L,128,64]
    if flags["affine"]:
        consts["lng_in"] = np.tile(ln_g[:, None, :], (1, 128, 1))     # [L-1,128,64]
        consts["lnb_in"] = np.tile(ln_b[:, None, :], (1, 128, 1))
    struct = {
        "blocks": blocks,            # [n_chunks, nt]
        "nblk_total": nblk_total,
        "e_pad": e_pad,
    }
    return per_core, consts, struct, flags


def _build(cfg, struct, flags):
    """Build the Bass/Tile program. Returns nc."""
    c = cfg
    blocks = struct["blocks"]
    nblk_total = struct["nblk_total"]
    e_pad = struct["e_pad"]
    D = c.D
    NT = c.nt
    HT = NT // 2                 # tiles per half
    MSG = c.msg_dt
    ROWE = c.row_elems

    # first chunk contributing each window (for copy-vs-add into agg)
    first_ck = [None] * NT
    for w in range(NT):
        for ck in range(c.n_chunks):
            if blocks[ck, w] > 0:
                first_ck[w] = ck
                break

    nc = bacc.Bacc("TRN2", num_devices=c.n_cores, target_bir_lowering=False,
                   debug=False, enable_asserts=False,
                   num_swdge_queues=c.n_queues,
                   dynamic_dma_scratch_size=c.dma_scratch)

    # I/O
    x_in = nc.dram_tensor("x_in", [c.t_pad, D], F32, kind="ExternalInput")
    dinv_in = nc.dram_tensor("dinv_in", [128, NT], F32, kind="ExternalInput")
    idxs_in = nc.dram_tensor("idxs_in", [128, e_pad // 16], I16, kind="ExternalInput")
    rreo_in = nc.dram_tensor("rreo_in", [128, 2 * nblk_total], MSG, kind="ExternalInput")
    iota_in = nc.dram_tensor("iota_in", [128, 2 * c.sblk * 128], MSG, kind="ExternalInput")
    W_in = nc.dram_tensor("W_in", [c.L, D, D], F32, kind="ExternalInput")
    i64_in = nc.dram_tensor("i64_in", [64, 64], F32, kind="ExternalInput")
    i128_in = nc.dram_tensor("i128_in", [128, 128], F32, kind="ExternalInput")
    if flags["bias"]:
        bs_in = nc.dram_tensor("bs_in", [c.L, 128, D], F32, kind="ExternalInput")
    if flags["affine"]:
        lng_in = nc.dram_tensor("lng_in", [c.L - 1, 128, D], F32, kind="ExternalInput")
        lnb_in = nc.dram_tensor("lnb_in", [c.L - 1, 128, D], F32, kind="ExternalInput")
    emb_out = nc.dram_tensor("emb_out", [c.t_pad, D], F32, kind="ExternalOutput")
    x_out = nc.dram_tensor("x_out", [c.t_pad, D], F32, kind="ExternalOutput")

    with tile.TileContext(nc) as tc:
        with (
            tc.tile_pool(name="dram", bufs=1, space="DRAM") as dram_pool,
            tc.tile_pool(name="const", bufs=1) as const_pool,
            tc.tile_pool(name="state", bufs=1) as state_pool,
            tc.tile_pool(name="xT", bufs=2) as xT_pool,
            tc.tile_pool(name="msg", bufs=8) as msg_pool,
            tc.tile_pool(name="iseg", bufs=4) as iseg_pool,
            tc.tile_pool(name="sS", bufs=8) as s_pool,
            tc.tile_pool(name="stats", bufs=2) as stats_pool,
            tc.tile_pool(name="pagg", bufs=4, space="PSUM") as pagg_pool,
            tc.tile_pool(name="pbig", bufs=2, space="PSUM") as pbig_pool,
            tc.tile_pool(name="ptr", bufs=2, space="PSUM") as ptr_pool,
        ):
            # ---- DRAM internal buffers: packed bf16 shard + gathered tables
            hp_shards = [
                dram_pool.tile([c.t_pad, D], MSG, name=f"hp_shard{i}")
                for i in range(c.L)]
            hp_fulls = [
                [dram_pool.tile([c.chunk_pairs, ROWE], MSG, addr_space="Shared",
                                name=f"hp_full{i}q{q}")
                 for q in range(c.n_chunks)]
                for i in range(c.L)]

            # ---- constants ----
            dinvT = const_pool.tile([128, NT], F32)
            nc.sync.dma_start(dinvT[:], dinv_in[:])
            rrEO = const_pool.tile([128, 2 * nblk_total], MSG)
            nc.sync.dma_start(rrEO[:], rreo_in[:])
            iota16 = const_pool.tile([128, 2 * c.sblk, 128], MSG)
            nc.sync.dma_start(iota16[:], iota_in[:].rearrange("p (s k) -> p s k", k=128))
            i64 = const_pool.tile([64, 64], F32)
            nc.sync.dma_start(i64[:], i64_in[:])
            i128 = const_pool.tile([128, 128], F32)
            nc.sync.dma_start(i128[:], i128_in[:])
            eps_sb = const_pool.tile([128, 1], F32)
            nc.vector.memset(eps_sb[:], float(c.eps))
            W_sb = const_pool.tile([64, c.L, D], F32)
            nc.sync.dma_start(W_sb[:], W_in[:].rearrange("l p j -> p l j"))
            if flags["bias"]:
                bs_sb = const_pool.tile([128, c.L, D], F32)
                nc.sync.dma_start(bs_sb[:], bs_in[:].rearrange("l p j -> p l j"))
            if flags["affine"]:
                lng_sb = const_pool.tile([128, c.L - 1, D], F32)
                nc.sync.dma_start(lng_sb[:], lng_in[:].rearrange("l p j -> p l j"))
                lnb_sb = const_pool.tile([128, c.L - 1, D], F32)
                nc.sync.dma_start(lnb_sb[:], lnb_in[:].rearrange("l p j -> p l j"))

            # ---- persistent state ----
            x_state = state_pool.tile([128, NT, D], F32)
            hp_own = state_pool.tile([128, NT, D], F32)
            hp_bf = state_pool.tile([128, NT, D], MSG)
            agg = state_pool.tile([128, NT, D], F32)

            nc.sync.dma_start(x_state[:], x_in[:].rearrange("(t p) f -> p t f", p=128))

            dinv_bc = dinvT[:].broadcast_to([128, NT, D])

            gq = [0]  # round-robin gather queue counter
            pending_ags = []  # deferred collective dispatches [fn, fire_at]

            def emit_linear(layer, h):
                """h' = (x @ W) * dinv for half h; cast, store, all-gather."""
                t0 = h * HT
                st = t0
                while st < t0 + HT:
                    n_t = min(4, t0 + HT - st)
                    xdT = xT_pool.tile([64, 4, 128], F32, tag="xdT")
                    for j in range(n_t):
                        t = st + j
                        ptr = ptr_pool.tile([64, 128], F32, tag="ptr")
                        nc.tensor.transpose(ptr[:], x_state[:, t, :], i128[:])
                        nc.scalar.copy(xdT[:, j, :], ptr[:])
                    hT_ps = pbig_pool.tile([64, 4 * 128], F32)
                    nc.tensor.matmul(
                        hT_ps[:, :n_t * 128],
                        W_sb[:, layer, :],
                        xdT[:, :n_t, :],
                        start=True, stop=True)
                    hT_sb = xT_pool.tile([64, 4, 128], F32, tag="hT")
                    nc.scalar.copy(
                        hT_sb[:, :n_t, :],
                        hT_ps[:, :n_t * 128].rearrange("p (a b) -> p a b", b=128))
                    for j in range(n_t):
                        t = st + j
                        ptr2 = ptr_pool.tile([128, 64], F32, tag="ptr")
                        nc.tensor.transpose(ptr2[:], hT_sb[:, j, :], i64[:])
                        nc.vector.tensor_scalar(
                            hp_own[:, t, :], ptr2[:], dinvT[:, t:t + 1], None,
                            AOP.mult)
                    st += n_t
                nc.vector.tensor_copy(
                    hp_bf[:, t0:t0 + HT, :], hp_own[:, t0:t0 + HT, :])
                nc.sync.dma_start(
                    hp_shards[layer][:].rearrange(
                        "(t p) f -> p t f", p=128)[:, t0:t0 + HT, :],
                    hp_bf[:, t0:t0 + HT, :])
                def _ag(layer=layer, h=h):
                    nc.gpsimd.collective_compute(
                        "AllGather", AOP.bypass,
                        replica_groups=[list(range(c.n_cores))],
                        ins=[hp_shards[layer][
                            h * c.half_nodes:(h + 1) * c.half_nodes, :].opt()],
                        outs=[hp_fulls[layer][h][:].opt()],
                    )
                if layer == 0 and h == 0:
                    _ag()          # nothing ahead of it in the Pool queue
                else:
                    pending_ags.append([_ag, gq[0] + 6])

            def emit_epilogue(layer, h):
                """skip add + dinv scale + relu (+LN) for half h windows."""
                t0, t1 = h * HT, (h + 1) * HT
                a = agg[:, t0:t1, :]
                xs = x_state[:, t0:t1, :]
                db = dinvT[:, t0:t1].broadcast_to([128, HT, D])
                nc.vector.tensor_tensor(a, a, hp_own[:, t0:t1, :], AOP.add)
                nc.vector.tensor_tensor(a, a, db, AOP.mult)
                nc.vector.tensor_tensor(xs, xs, a, AOP.add)
                if flags["bias"]:
                    for t in range(t0, t1):
                        nc.vector.tensor_tensor(
                            x_state[:, t, :], x_state[:, t, :],
                            bs_sb[:, layer, :], AOP.add)
                if layer == c.L - 1:
                    # emb = x (pre-relu); x_out = relu(emb)
                    nc.sync.dma_start(
                        emb_out[:].rearrange(
                            "(t p) f -> p t f", p=128)[:, t0:t1, :], xs)
                    nc.vector.tensor_scalar(a, xs, 0.0, None, AOP.max)
                    nc.sync.dma_start(
                        x_out[:].rearrange(
                            "(t p) f -> p t f", p=128)[:, t0:t1, :], a)
                else:
                    nc.vector.tensor_scalar(xs, xs, 0.0, None, AOP.max)
                    mu = stats_pool.tile([128, HT], F32, tag="mu")
                    ss = stats_pool.tile([128, HT], F32, tag="ss")
                    rstd = stats_pool.tile([128, HT], F32, tag="rstd")
                    vtmp = stats_pool.tile([128, HT], F32, tag="vtmp")
                    nc.vector.tensor_reduce(mu[:], xs, mybir.AxisListType.X, AOP.add)
                    nc.scalar.activation(a, xs, mybir.ActivationFunctionType.Square)
                    nc.vector.tensor_reduce(ss[:], a, mybir.AxisListType.X, AOP.add)
                    nc.vector.tensor_scalar(mu[:], mu[:], 1.0 / D, None, AOP.mult)
                    nc.vector.tensor_tensor(vtmp[:], mu[:], mu[:], AOP.mult)
                    nc.vector.scalar_tensor_tensor(
                        vtmp[:], ss[:], 1.0 / D, vtmp[:], AOP.mult, AOP.subtract)
                    nc.scalar.activation(vtmp[:], vtmp[:],
                                         mybir.ActivationFunctionType.Sqrt,
                                         bias=eps_sb[:])
                    nc.vector.reciprocal(rstd[:], vtmp[:])
                    nc.vector.tensor_tensor(
                        xs, xs, mu[:].broadcast_to([128, HT, D]), AOP.subtract)
                    nc.vector.tensor_tensor(
                        xs, xs, rstd[:].broadcast_to([128, HT, D]), AOP.mult)
                    if flags["affine"]:
                        for t in range(t0, t1):
                            nc.vector.tensor_tensor(
                                x_state[:, t, :], x_state[:, t, :],
                                lng_sb[:, layer, :], AOP.mult)
                            nc.vector.tensor_tensor(
                                x_state[:, t, :], x_state[:, t, :],
                                lnb_sb[:, layer, :], AOP.add)

            nidx_regs = {}

            emit_linear(0, 0)
            emit_linear(0, 1)

            for layer in range(c.L):
                # windows with no edges at all
                for w in range(NT):
                    if first_ck[w] is None:
                        nc.vector.memset(agg[:, w, :], 0.0)

                # ---- gather + one-hot matmul reduction ----
                gb = 0               # global block id
                sEO = None
                ps = None
                done_h0 = False

                def boundary_work():
                    emit_epilogue(layer, 0)
                    if layer < c.L - 1:
                        emit_linear(layer + 1, 0)

                for ck in range(c.n_chunks):
                    ck_blocks = []   # (w, idx_in_group, group_size)
                    for w in range(NT):
                        for i in range(int(blocks[ck, w])):
                            ck_blocks.append((w, i, int(blocks[ck, w])))
                    nb_ck = len(ck_blocks)
                    if nb_ck == 0:
                        continue
                    in_rows = hp_fulls[layer][ck][:]
                    n_seg = (nb_ck + c.segblk - 1) // c.segblk
                    gb0_ck = gb
                    for s in range(n_seg):
                        b0 = s * c.segblk
                        nblk_s = min(c.segblk, nb_ck - b0)
                        nidx = nblk_s * 128
                        goff = (gb0_ck + b0) * 8   # idx cols (8 per 128 slots)
                        iseg = iseg_pool.tile([128, c.segblk * 8], I16)
                        nc.sync.dma_start(
                            iseg[:, :nblk_s * 8],
                            idxs_in[:, goff:goff + nblk_s * 8])
                        msg = msg_pool.tile([128, c.segblk, ROWE], MSG)
                        if nidx not in nidx_regs:
                            nidx_regs[nidx] = nc.gpsimd.to_reg(nidx)
                        nc.gpsimd.dma_gather(
                            msg[:, :nblk_s, :], in_rows,
                            iseg[:, :nblk_s * 8],
                            nidx, nidx_regs[nidx], ROWE,
                            single_packet=(nidx <= 1024 or c.force_single_packet),
                            queue_num=gq[0] % c.n_queues)
                        gq[0] += 1
                        while pending_ags and gq[0] >= pending_ags[0][1]:
                            pending_ags.pop(0)[0]()
                        for bl in range(nblk_s):
                            w, gi, gsz = ck_blocks[b0 + bl]
                            # h0 windows all finalized -> overlap boundary work
                            if ck == c.n_chunks - 1 and w >= HT and not done_h0:
                                boundary_work()
                                done_h0 = True
                            g = gb + b0 + bl
                            # build S tiles (even|odd interleaved) per sblk
                            if g % c.sblk == 0:
                                k = min(c.sblk, nblk_total - g)
                                sEO = s_pool.tile([128, c.sblk, 2, 128], MSG,
                                                  tag="SEO")
                                nc.vector.tensor_tensor(
                                    sEO[:, :k, :, :].rearrange("p a b c -> p (a b) c"),
                                    iota16[:, :2 * k, :],
                                    rrEO[:, 2 * g:2 * (g + k)].broadcast_to(
                                        [128, 2 * k, 128]),
                                    AOP.is_equal)
                            if gi == 0:
                                ps = pagg_pool.tile([128, D], F32)
                            nc.tensor.matmul(
                                ps[:], sEO[:, g % c.sblk, 0, :],
                                msg[:, bl, 0:64],
                                start=(gi == 0), stop=False)
                            nc.tensor.matmul(
                                ps[:], sEO[:, g % c.sblk, 1, :],
                                msg[:, bl, 64:128],
                                start=False, stop=(gi == gsz - 1))
                            if gi == gsz - 1:
                                if first_ck[w] == ck:
                                    nc.scalar.copy(agg[:, w, :], ps[:])
                                else:
                                    nc.vector.tensor_tensor(
                                        agg[:, w, :], agg[:, w, :], ps[:], AOP.add)
                    gb += nb_ck

                while pending_ags:
                    pending_ags.pop(0)[0]()
                if not done_h0:
                    boundary_work()
                emit_epilogue(layer, 1)
                if layer < c.L - 1:
                    emit_linear(layer + 1, 1)

    nc.compile()
    return nc


_CACHE = {}
last_results = None


def _run(cfg, inputs, trace=False):
    global last_results
    per_core, consts, struct, flags = _prep(cfg, **inputs)
    key = (cfg.N, cfg.E, cfg.seg_edges, cfg.n_queues, struct["nblk_total"])
    if key not in _CACHE:
        _CACHE[key] = _build(cfg, struct, flags)
    nc = _CACHE[key]
    in_maps = []
    for cc in range(cfg.n_cores):
        m = dict(consts)
        m.update(per_core[cc])
        in_maps.append(m)
    res = run_bass_kernel_spmd(nc, in_maps, list(range(cfg.n_cores)), trace=trace)
    last_results = res
    emb = np.concatenate(
        [np.asarray(r["emb_out"])[:cfg.npc] for r in res.results], axis=0)
    xf = np.concatenate(
        [np.asarray(r["x_out"])[:cfg.npc] for r in res.results], axis=0)
    return emb, xf


def kernel(x, edge_index, Ws, bs, ln_g, ln_b):
    cfg = Cfg(seg_edges=int(os.environ.get("GCN_SEG", "3072")),
              dma_scratch=int(os.environ.get("GCN_SCRATCH", "16384")),
              n_queues=int(os.environ.get("GCN_NQ", "4")),
              force_single_packet=os.environ.get("GCN_SP", "0") == "1")
    return _run(cfg, dict(x=x, edge_index=edge_index, Ws=Ws, bs=bs,
                          ln_g=ln_g, ln_b=ln_b),
                trace=os.environ.get("GCN_TRACE", "0") == "1")
